# revision 57
# baseline (speedup 1.0000x reference)
"""NativeSparseAttention Trainium2 kernel (8-core SPMD), v3.

Sharding: core c handles (b, kv) = (c // 4, c % 4); all three attention
branches, the gate/compress MLPs, and the k/v projections for that
(batch, kv-head) pair are fully independent across cores.

Numerics (same plan as the validated baseline):
  - branch-1 chain (k_cmp projection, compress MLP, branch-1 scores,
    softmax for p_grp, top-16 selection) in fp32; exp via a degree-6
    polynomial P(s) ~ exp(s/2) squared (rel err ~7e-7; the ACT LUT exp
    is only ~1e-5 and block selection needs ~1e-6 to keep the fp32
    reference's top-k ordering).
  - branches 2/3 in bf16 on the PE with fp32 PSUM accumulation;
    branch outputs held in bf16, combined with fp32 accumulation.
  - softmax skips max-subtraction (live score range is small at this
    model's scale) and normalizes after the PV matmul via an appended
    ones-column in V (row-sum lands in output column 128).

Scheduling: slc/win projections run first; branch-3 and branch-2
score/exp work is emitted between chunks of the fp32 k_cmp projection
so ACT/DVE overlap the PE-heavy phase; bf16 elementwise sits on DVE
(2x mode); every DMA is contiguous per partition and issued from the
SP queue (gpsimd-issued DMAs consume Pool engine time).
"""

import sys
import os

USE_LUT_EXP = os.environ.get("NSA_LUT", "0") == "1"

for _p in ("/opt/trn_rl_repo", "/root/.axon_site/_ro/trn_rl_repo"):
    if _p not in sys.path:
        sys.path.append(_p)

import numpy as np
import ml_dtypes

import concourse.bass as bass
import concourse.mybir as mybir
import concourse.tile as tile
from concourse import bacc
from concourse.bass_utils import run_bass_kernel_spmd

AF = mybir.ActivationFunctionType
ALU = mybir.AluOpType
F32 = mybir.dt.float32
BF16 = mybir.dt.bfloat16

B, T, DM = 2, 1024, 2048
NQ, NKV, DH = 16, 4, 128
BLK, STRIDE, TOPN, WIN = 32, 16, 16, 512
NREP = NQ // NKV
NB = 63
NBP = 64                    # padded block count (col 63 is dead)
MO = DM // 128
TB = T // 128
TC = T // 512
SCALE = DH ** -0.5
STARTS = np.minimum(np.arange(NB) * STRIDE, T - 1)
NCORES = 8

# degree-6 fit of exp(s/2) on |s| <= 1.3; exp(s) = P(s)^2, rel err ~7e-7
_xs = np.cos(np.pi * (np.arange(8000) + 0.5) / 8000) * 1.3
_V = np.vander(_xs, 7, increasing=True)
EXPC = [float(v) for v in np.linalg.lstsq(_V, np.exp(_xs / 2), rcond=None)[0]]


def _emit(nc, tc, d, out_dram):
    def ap(name):
        return d[name].ap()

    from contextlib import ExitStack
    stk = ExitStack()
    consts = stk.enter_context(tc.tile_pool(name="consts", bufs=1))
    pers = stk.enter_context(tc.tile_pool(name="pers", bufs=1))
    pp = stk.enter_context(tc.tile_pool(name="pp", bufs=2, space="PSUM"))
    psS = stk.enter_context(tc.tile_pool(name="psS", bufs=2, space="PSUM"))
    psV = stk.enter_context(tc.tile_pool(name="psV", bufs=2, space="PSUM"))
    psA = stk.enter_context(tc.tile_pool(name="psA", bufs=1, space="PSUM"))

    # ---------------- persistent tiles ----------------
    qb_sb = pers.tile([128, NREP, T], BF16, tag="qb")
    kslcT = pers.tile([128, T], BF16, tag="kslcT")
    kwinT = pers.tile([128, T], BF16, tag="kwinT")
    vslc = pers.tile([128, TB, 129], BF16, tag="vslc")
    vwin = pers.tile([128, TB, 129], BF16, tag="vwin")
    nc.vector.memset(vslc[:, :, 128:129], 1.0)
    nc.vector.memset(vwin[:, :, 128:129], 1.0)
    kcmpT = pers.tile([128, T], F32, tag="kcmpT")
    vcmpT = pers.tile([128, T], BF16, tag="vcmpT")
    gates = pers.tile([128, TB, 12], F32, tag="gates")
    ksumT = pers.tile([128, NBP], F32, tag="ksumT")
    ksum_bf = pers.tile([128, NBP], BF16, tag="ksumbf")
    vsuma_bf = pers.tile([NBP, 129], BF16, tag="vsumabf")
    o_win = pers.tile([128, TB, NREP, 129], BF16, tag="owin")
    h_k = pers.tile([128, NBP], F32, tag="hk")
    h_v = pers.tile([128, NBP], BF16, tag="hv")
    qpool = stk.enter_context(tc.tile_pool(name="qpool", bufs=1))
    # e2 lives through B..G; layout groups the head dim so one m01
    # multiply covers all four heads via a broadcast middle dim
    e2a = stk.enter_context(tc.tile_pool(name="e2a", bufs=1))
    e2 = e2a.tile([128, 12, NREP, 512], BF16, tag="e2")

    # ---------------- consts (sync DMA queue, ordered by first use) -------
    identb = consts.tile([128, 128], BF16, tag="identb")
    identf = consts.tile([128, 128], F32, tag="identf")
    caus01 = consts.tile([128, 128], BF16, tag="caus01")
    win01 = consts.tile([128, 128], BF16, tag="win01")
    tib_sb = consts.tile([NBP, T], BF16, tag="tib")
    maskA = consts.tile([128, TB, NBP], F32, tag="maskA")
    maskT01 = consts.tile([NBP, T], BF16, tag="maskT01")
    gw_sb = consts.tile([128, MO, 12], BF16, tag="gw")
    gbr = consts.tile([1, 12], BF16, tag="gbr")
    onesb = consts.tile([1, 128], BF16, tag="onesb")
    brv = consts.tile([1, 129], BF16, tag="brv")
    b1k = consts.tile([128, 1], F32, tag="b1k")
    b1v = consts.tile([128, 1], F32, tag="b1v")
    ck2_sb = consts.tile([128, 128], F32, tag="ck2")
    ck2b = consts.tile([128, 1], F32, tag="ck2b")
    cv2a = consts.tile([128, 129], BF16, tag="cv2a")
    c = EXPC
    c0b = consts.tile([128, 1], F32, tag="c0b")
    c4b = consts.tile([128, 1], F32, tag="c4b")
    nc.vector.memset(c0b[:], c[0])
    nc.vector.memset(c4b[:], c[4])
    nc.vector.memset(onesb[:], 1.0)

    # ================= stage A: slc/win projections =================
    epool = stk.enter_context(tc.tile_pool(name="epool", bufs=1))
    stkAD = ExitStack()
    projp = stkAD.enter_context(tc.tile_pool(name="projp", bufs=1))
    trig = stkAD.enter_context(tc.tile_pool(name="trig", bufs=1))
    wstrm = stkAD.enter_context(tc.tile_pool(name="wstrm", bufs=2))
    ev = stkAD.enter_context(tc.tile_pool(name="ev", bufs=2))

    xb_sb = projp.tile([128, MO, T], BF16, tag="xb")
    w0_sb = wstrm.tile([128, MO, 128], BF16, tag="wcur")
    nc.sync.dma_start(w0_sb[:], ap("wTb")[0])
    nc.sync.dma_start(xb_sb[:, :, 0:512], ap("xTb")[:, :, 0:512])
    cosb = trig.tile([64, T], BF16, tag="cosb")
    sinb = trig.tile([64, T], BF16, tag="sinb")
    cosf = trig.tile([64, T], F32, tag="cosf")
    sinf = trig.tile([64, T], F32, tag="sinf")
    nc.sync.dma_start(cosb[:], ap("cosb"))
    nc.sync.dma_start(sinb[:], ap("sinb"))
    nc.sync.dma_start(xb_sb[:, :, 512:T], ap("xTb")[:, :, 512:T])

    def rope_bf(ps, tck, outT):
        # bf16 rotate-half rope from psum [128, 512] into outT[128, T] slice
        sl = slice(tck * 512, (tck + 1) * 512)
        tlo = ev.tile([64, 512], BF16, tag="tlo")
        thi = ev.tile([64, 512], BF16, tag="thi")
        nc.scalar.copy(tlo[:], ps[0:64, :])
        nc.scalar.copy(thi[:], ps[64:128, :])
        ta = ev.tile([64, 512], BF16, tag="ropa")
        cc = cosb[:, sl]
        ss = sinb[:, sl]
        nc.vector.tensor_tensor(outT[0:64, sl], tlo[:], cc, op=ALU.mult)
        nc.vector.tensor_tensor(ta[:], thi[:], ss, op=ALU.mult)
        nc.vector.tensor_sub(outT[0:64, sl], outT[0:64, sl], ta[:])
        nc.vector.tensor_tensor(outT[64:128, sl], tlo[:], ss, op=ALU.mult)
        nc.vector.tensor_tensor(ta[:], thi[:], cc, op=ALU.mult)
        nc.vector.tensor_add(outT[64:128, sl], outT[64:128, sl], ta[:])

    def v_evict(ps, tck, vdst):
        tmp = ev.tile([128, 512], BF16, tag="vtmp")
        nc.scalar.copy(tmp[:], ps[:])
        for j in range(4):
            kt = tck * 4 + j
            pst = psA.tile([128, 128], BF16, tag="Xb")
            nc.tensor.transpose(pst[:], tmp[:, j * 128:(j + 1) * 128], identb[:])
            nc.vector.tensor_copy(vdst[:, kt, 0:128], pst[:])

    for wi, (kind, dst) in enumerate((("k", kwinT), ("v", vwin),
                                      ("k", kslcT), ("v", vslc))):
        if wi == 0:
            w_wi = w0_sb
        else:
            w_wi = wstrm.tile([128, MO, 128], BF16, tag="wcur")
            nc.sync.dma_start(w_wi[:], ap("wTb")[wi])
        if wi == 1:
            nc.sync.dma_start(qb_sb[:], ap("qTb"))
        for tck in range(TC):
            ps = pp.tile([128, 512], F32, tag="P")
            for mo in range(MO):
                nc.tensor.matmul(ps[:], w_wi[:, mo, :],
                                 xb_sb[:, mo, tck * 512:(tck + 1) * 512],
                                 start=(mo == 0), stop=(mo == MO - 1))
            if kind == "k":
                rope_bf(ps, tck, dst)
            else:
                v_evict(ps, tck, dst)

    for t_, n_ in ((caus01, "caus01"), (win01, "win01"), (cosf, "cosf"),
                   (sinf, "sinf"), (gw_sb, "gw"), (gbr, "gbr"),
                   (identb, "identb"), (identf, "identf")):
        nc.sync.dma_start(t_[:], ap(n_))

    # gates: [t, 12] per tb
    for tb in range(TB):
        ps = psA.tile([128, 129], F32, tag="X")
        for mo in range(MO):
            nc.tensor.matmul(ps[:, 0:12], xb_sb[:, mo, tb * 128:(tb + 1) * 128],
                             gw_sb[:, mo, :], start=(mo == 0), stop=False)
        nc.tensor.matmul(ps[:, 0:12], onesb[:], gbr[:], start=False, stop=True)
        nc.scalar.activation(gates[:, tb, :], ps[:, 0:12], AF.Sigmoid)

    # ================= stage B: fp32 k_cmp proj + branch 3 + b2 sc ========
    xstrm = stkAD.enter_context(tc.tile_pool(name="xstrm", bufs=2))
    wcp = projp.tile([128, MO, 128], F32, tag="wcmp")
    nc.sync.dma_start(wcp[:], ap("wcmp"))

    def rope_f32(ps, ch, eng):
        sl = slice(ch * 128, (ch + 1) * 128)
        cc = cosf[:, sl]
        ss = sinf[:, sl]
        ta = ev.tile([64, 128], F32, tag="fra")
        tb_ = ev.tile([64, 128], F32, tag="frb")
        eng.tensor_tensor(kcmpT[0:64, sl], ps[0:64, :], cc, op=ALU.mult)
        eng.tensor_tensor(ta[:], ps[64:128, :], ss, op=ALU.mult)
        eng.tensor_sub(kcmpT[0:64, sl], kcmpT[0:64, sl], ta[:])
        eng.tensor_tensor(kcmpT[64:128, sl], ps[0:64, :], ss, op=ALU.mult)
        eng.tensor_tensor(tb_[:], ps[64:128, :], cc, op=ALU.mult)
        eng.tensor_add(kcmpT[64:128, sl], kcmpT[64:128, sl], tb_[:])

    def emit_b3(g):
        for i in range(TB):
            sl = slice(i * 128, (i + 1) * 128)
            kts = list(range(max(0, i - 4), i + 1))
            groups = [kts[j:j + 4] for j in range(0, len(kts), 4)]
            e3 = {}
            for grp in groups:
                ps = psS.tile([128, 512], F32, tag="S")
                for j, kt in enumerate(grp):
                    nc.tensor.matmul(ps[:, j * 128:(j + 1) * 128],
                                     kwinT[:, kt * 128:(kt + 1) * 128],
                                     qb_sb[:, g, sl], start=True, stop=True)
                et = epool.tile([128, 4, 128], BF16, tag=f"e3g{grp[0] % 3}")
                nc.scalar.activation(
                    et[:, 0:len(grp), :],
                    ps[:, 0:len(grp) * 128].rearrange("p (a b) -> p a b", b=128),
                    AF.Exp)
                for j, kt in enumerate(grp):
                    if kt == i:
                        nc.vector.tensor_tensor(et[:, j, :], et[:, j, :],
                                                caus01[:], op=ALU.mult)
                    elif kt == i - 4:
                        nc.vector.tensor_tensor(et[:, j, :], et[:, j, :],
                                                win01[:], op=ALU.mult)
                    e3[kt] = et[:, j, :]
            psv = psV.tile([128, 129], F32, tag="V")
            for kt in kts:
                nc.tensor.matmul(psv[:], e3[kt], vwin[:, kt, :],
                                 start=(kt == kts[0]), stop=(kt == kts[-1]))
            nc.gpsimd.tensor_copy(o_win[:, i, g, :], psv[:])

    def emit_b2sc(g):
        # branch-2 scores + exp for head-group g into e2[:, j, g, :]
        for tck in range(TC):
            for kt in range(4 * tck + 4):
                j = kt if tck == 0 else 4 + kt
                qs = max(0, kt * 128 - tck * 512)
                ps = psS.tile([128, 512], F32, tag="S")
                nc.tensor.matmul(ps[:, qs:512], kslcT[:, kt * 128:(kt + 1) * 128],
                                 qb_sb[:, g, tck * 512 + qs:(tck + 1) * 512],
                                 start=True, stop=True)
                nc.scalar.activation(e2[:, j, g, qs:512], ps[:, qs:512], AF.Exp)

    for qtr in range(4):
        for hf in range(2):
            ch = qtr * 2 + hf
            ps = pp.tile([128, 512], F32, tag="P")
            for moh in range(2):
                xq = xstrm.tile([128, 8, 128], F32, tag="xq")
                nc.sync.dma_start(xq[:], ap("xTq")[ch, :, moh * 8:(moh + 1) * 8])
                for mo in range(8):
                    nc.tensor.matmul(ps[:, 0:128], wcp[:, moh * 8 + mo, :],
                                     xq[:, mo, :],
                                     start=(moh == 0 and mo == 0),
                                     stop=(moh == 1 and mo == 7))
            rope_f32(ps[:, 0:128], ch,
                     nc.vector if qtr % 2 == 0 else nc.gpsimd)
        emit_b3(qtr)
        if qtr >= 1:
            emit_b2sc(qtr - 1)

    # ================= stage C: v_cmp projection =================
    q_sb = qpool.tile([128, NREP, T], F32, tag="q")
    nc.sync.dma_start(q_sb[:], ap("qT"))
    wvc = wstrm.tile([128, MO, 128], BF16, tag="wcur")
    nc.sync.dma_start(wvc[:], ap("wvcmp"))
    for tck in range(TC):
        ps = pp.tile([128, 512], F32, tag="P")
        for mo in range(MO):
            nc.tensor.matmul(ps[:], wvc[:, mo, :],
                             xb_sb[:, mo, tck * 512:(tck + 1) * 512],
                             start=(mo == 0), stop=(mo == MO - 1))
        nc.scalar.copy(vcmpT[:, tck * 512:(tck + 1) * 512], ps[:])
    emit_b2sc(3)

    # ================= stage D: compress MLPs =================
    for t_, n_ in ((maskA, "maskA"), (maskT01, "maskT01"), (tib_sb, "tib"),
                   (b1k, "b1k"), (b1v, "b1v"), (ck2_sb, "ck2"),
                   (ck2b, "ck2b"), (cv2a, "cv2a"), (brv, "brv")):
        nc.sync.dma_start(t_[:], ap(n_))
    for wn, srcT, bias1, h, dt_ in (("ck1_wT", kcmpT, b1k, h_k, F32),
                                    ("cv1_wT", vcmpT, b1v, h_v, BF16)):
        ps = pp.tile([128, 512], F32, tag="P")
        for cg in range(2):
            w1c = wstrm.tile([128, 16, 128], dt_, tag=f"w1c{dt_}", bufs=1)
            nc.sync.dma_start(w1c[:], ap(wn)[:, cg * 16:(cg + 1) * 16, :])
            for cc in range(16):
                c_ = cg * 16 + cc
                rhs = srcT[:, c_:c_ + 16 * (NB - 1) + 1:16]
                nc.tensor.matmul(ps[:, 0:NB], w1c[:, cc, :], rhs,
                                 start=(c_ == 0), stop=(c_ == BLK - 1))
        nc.vector.memset(h[:, NB:NBP], 0.0)
        nc.scalar.activation(h[:, 0:NB], ps[:, 0:NB], AF.Gelu, bias=bias1[:])

    ps = pp.tile([128, 512], F32, tag="P")
    nc.tensor.matmul(ps[:, 0:NBP], ck2_sb[:], h_k[:], start=True, stop=True)
    nc.scalar.activation(ksumT[:], ps[:, 0:NBP], AF.Identity, bias=ck2b[:])
    nc.vector.tensor_copy(ksum_bf[:], ksumT[:])

    ps = psA.tile([128, 129], F32, tag="X")
    nc.tensor.matmul(ps[0:NBP, :], h_v[:], cv2a[:], start=True, stop=False)
    nc.tensor.matmul(ps[0:NBP, :], onesb[:, 0:NBP], brv[:], start=False, stop=True)
    nc.vector.tensor_copy(vsuma_bf[:], ps[0:NBP, :])

    # ================= stage E: branch 1 (poly softmax) =================
    stkAD.close()
    pers2 = stk.enter_context(tc.tile_pool(name="pers2", bufs=1))
    pgrp = pers2.tile([128, TB, NBP], F32, tag="pgrp")
    m01 = pers2.tile([128, TB, T], BF16, tag="m01")
    o_cmp = pers2.tile([128, TB, NREP, 129], BF16, tag="ocmp")
    o_slc = pers2.tile([128, TB, NREP, 129], BF16, tag="oslc")
    selT = pers2.tile([NBP, T], BF16, tag="selT")
    polyp = stk.enter_context(tc.tile_pool(name="polyp", bufs=2))

    def emit_b1(g):
        pss = psS.tile([128, 512], F32, tag="S")
        for tb in range(TB):
            nc.tensor.matmul(pss[:, tb * 64:(tb + 1) * 64],
                             q_sb[:, g, tb * 128:(tb + 1) * 128],
                             ksumT[:], start=True, stop=True)
        eA = polyp.tile([128, TB, NBP], F32, tag="eA")
        S = polyp.tile([128, TB, 1], F32, tag="pS")
        r = polyp.tile([128, TB, 1], F32, tag="pr")
        halves = ((nc.vector, slice(0, 5)), (nc.gpsimd, slice(5, TB)))

        def tt(out, in0, in1, op=ALU.mult):
            for eng, hs in halves:
                eng.tensor_tensor(out[:, hs, :], in0[:, hs, :],
                                  in1[:, hs, :] if in1.shape[1] == TB else in1,
                                  op=op)

        if USE_LUT_EXP:
            s3 = pss[:].rearrange("p (a b) -> p a b", b=NBP)
            nc.scalar.activation(eA[:], s3, AF.Exp)
            tt(eA, eA, maskA)
        else:
            # P(s) = (c0+c1 s) + w(c2+c3 s) + v(c4+c5 s + c6 w); exp(s) = P^2
            sA = polyp.tile([128, TB, NBP], F32, tag="sA")
            nc.scalar.copy(sA[:].rearrange("p a b -> p (a b)"), pss[:])
            w_ = polyp.tile([128, TB, NBP], F32, tag="w")
            v_ = polyp.tile([128, TB, NBP], F32, tag="v")
            t1 = polyp.tile([128, TB, NBP], F32, tag="t1")
            t2 = polyp.tile([128, TB, NBP], F32, tag="t2")
            t3 = polyp.tile([128, TB, NBP], F32, tag="t3")
            nc.vector.tensor_tensor(w_[:], sA[:], sA[:], op=ALU.mult)
            nc.scalar.activation(t1[:], sA[:], AF.Identity, bias=c0b[:],
                                 scale=c[1])
            nc.vector.tensor_scalar(t2[:], sA[:], c[3], c[2],
                                    op0=ALU.mult, op1=ALU.add)
            nc.scalar.activation(t3[:], sA[:], AF.Identity, bias=c4b[:],
                                 scale=c[5])
            nc.vector.scalar_tensor_tensor(t3[:], w_[:], c[6], t3[:],
                                           op0=ALU.mult, op1=ALU.add)
            nc.gpsimd.tensor_tensor(v_[:], w_[:], w_[:], op=ALU.mult)
            tt(t2, w_, t2)
            tt(t1, t1, t2, op=ALU.add)
            tt(t3, v_, t3)
            tt(t1, t1, t3, op=ALU.add)
            tt(t1, t1, maskA)
            tt(eA, t1, t1)
        nc.vector.reduce_sum(S[:], eA[:], axis=mybir.AxisListType.X)
        nc.vector.reciprocal(r[:], S[:])
        # pgrp[:, tb, :] (+)= eA[:, tb, :] * r[tb]  (per-partition scalar)
        for tb in range(TB):
            if g == 0:
                nc.vector.tensor_scalar(pgrp[:, tb, :], eA[:, tb, :],
                                        r[:, tb, :], None, op0=ALU.mult)
            else:
                nc.vector.scalar_tensor_tensor(pgrp[:, tb, :], eA[:, tb, :],
                                               r[:, tb, :], pgrp[:, tb, :],
                                               op0=ALU.mult, op1=ALU.add)

        # branch-1 output path
        eTt = polyp.tile([NBP, T], BF16, tag="eT")
        for tck in range(TC):
            sl = slice(tck * 512, (tck + 1) * 512)
            ps = pp.tile([128, 512], F32, tag="P")
            nc.tensor.matmul(ps[0:NBP, :], ksum_bf[:], qb_sb[:, g, sl],
                             start=True, stop=True)
            nc.scalar.activation(eTt[:, sl], ps[0:NBP, :], AF.Exp)
            nc.vector.tensor_tensor(eTt[:, sl], eTt[:, sl], maskT01[:, sl],
                                    op=ALU.mult)
        for tb in range(TB):
            psv = psV.tile([128, 129], F32, tag="V")
            nc.tensor.matmul(psv[:], eTt[:, tb * 128:(tb + 1) * 128],
                             vsuma_bf[:], start=True, stop=True)
            nc.gpsimd.tensor_copy(o_cmp[:, tb, g, :], psv[:])

    for g in range(NREP):
        emit_b1(g)

    # ================= stage F: top-16 + coverage mask + e2 masking =======
    fpool = stk.enter_context(tc.tile_pool(name="fpool", bufs=2))
    for tb in range(TB):
        mx = fpool.tile([128, 8], F32, tag="mx")
        sw = fpool.tile([128, NBP], F32, tag="sw")
        nc.vector.max(mx[:], pgrp[:, tb, :])
        nc.vector.match_replace(sw[:], mx[:], pgrp[:, tb, :], 0.0)
        nc.vector.max(mx[:], sw[:])
        nc.vector.match_replace(sw[:], mx[:], sw[:], 0.0)
        nc.vector.tensor_tensor(sw[:], pgrp[:, tb, :], sw[:], op=ALU.is_gt)
        pst = psA.tile([128, 129], F32, tag="X")
        nc.tensor.transpose(pst[0:NBP, 0:128], sw[:], identf[:])
        nc.scalar.copy(selT[:, tb * 128:(tb + 1) * 128], pst[0:NBP, 0:128])

    # m01 per key tile sc, immediately mask e2 and run the PV matmuls for
    # query tile i == sc (all earlier key tiles are already masked)
    for sc in range(TB):
        for tck in range((sc * 128) // 512, TC):
            qs = max(0, sc * 128 - tck * 512)
            sl = slice(tck * 512 + qs, (tck + 1) * 512)
            ps = psS.tile([128, 512], F32, tag="S")
            nc.tensor.matmul(ps[:, qs:512], tib_sb[:, sc * 128:(sc + 1) * 128],
                             selT[:, sl], start=True, stop=True)
            nc.vector.tensor_scalar(m01[:, sc, sl], ps[:, qs:512], 0.0, None,
                                    op0=ALU.is_gt)
        nc.vector.tensor_tensor(m01[:, sc, sc * 128:(sc + 1) * 128],
                                m01[:, sc, sc * 128:(sc + 1) * 128],
                                caus01[:], op=ALU.mult)
        # mask e2 for this key tile: one broadcast op covers all four heads
        for tck in range((sc * 128) // 512, TC):
            j = sc if tck == 0 else 4 + sc
            qs = max(0, sc * 128 - tck * 512)
            sl = slice(tck * 512 + qs, (tck + 1) * 512)
            mb = m01[:, sc:sc + 1, sl].to_broadcast([128, NREP, 512 - qs])
            nc.vector.tensor_tensor(e2[:, j, :, qs:512],
                                    e2[:, j, :, qs:512], mb, op=ALU.mult)
        # branch-2 PV for query tile i == sc (keys kt <= sc all masked now)
        i = sc
        tck = i // 4
        lo = (i - 4 * tck) * 128
        for g in range(NREP):
            psv = psV.tile([128, 129], F32, tag="V")
            for kt in range(i + 1):
                j = kt if tck == 0 else 4 + kt
                nc.tensor.matmul(psv[:], e2[:, j, g, lo:lo + 128],
                                 vslc[:, kt, :], start=(kt == 0),
                                 stop=(kt == i))
            nc.gpsimd.tensor_copy(o_slc[:, i, g, :], psv[:])

    # ================= stage G: combine =================
    for g in range(NREP):
        # normalize + gate + combine:  acc = sum_j gate_j/Z_j * o_j
        w0 = fpool.tile([128, TB, 1], F32, tag="w0")
        w1 = fpool.tile([128, TB, 1], F32, tag="w1")
        w2 = fpool.tile([128, TB, 1], F32, tag="w2")
        for wj, o_un, jj in ((w0, o_cmp, 0), (w1, o_slc, 1), (w2, o_win, 2)):
            nc.vector.reciprocal(wj[:], o_un[:, :, g, 128:129])
            nc.vector.tensor_tensor(wj[:], wj[:],
                                    gates[:, :, 3 * g + jj:3 * g + jj + 1],
                                    op=ALU.mult)
        accb = fpool.tile([128, TB, 128], BF16, tag="accb")
        accb2 = fpool.tile([128, TB, 128], BF16, tag="accb2")
        accf = fpool.tile([128, TB, 128], F32, tag="accf")
        for tb in range(TB):
            nc.scalar.activation(accb[:, tb, :], o_cmp[:, tb, g, 0:128],
                                 AF.Identity, scale=w0[:, tb, :])
            nc.vector.tensor_scalar(accb2[:, tb, :], o_slc[:, tb, g, 0:128],
                                    w1[:, tb, :], None, op0=ALU.mult)
        nc.vector.tensor_add(accb[:], accb[:], accb2[:])
        for tb in range(TB):
            nc.vector.scalar_tensor_tensor(accf[:, tb, :], o_win[:, tb, g, 0:128],
                                           w2[:, tb, :], accb[:, tb, :],
                                           op0=ALU.mult, op1=ALU.add)
        nc.sync.dma_start(out_dram.ap()[g].rearrange("(tb p) d -> p tb d", p=128),
                          accf[:])

    stk.close()


def _build_program():
    nc = bacc.Bacc("TRN2", target_bir_lowering=False, debug=False,
                   num_devices=NCORES)
    dram = {}

    def din(name, shape, dtype=F32):
        dram[name] = nc.dram_tensor(name, list(shape), dtype, kind="ExternalInput")

    din("xTq", (8, 128, MO, 128))
    din("xTb", (128, MO, T), BF16)
    din("qT", (128, NREP, T))
    din("qTb", (128, NREP, T), BF16)
    din("wcmp", (128, MO, 128))
    din("wvcmp", (128, MO, 128), BF16)
    din("wTb", (4, 128, MO, 128), BF16)
    din("gw", (128, MO, 12), BF16)
    din("gbr", (1, 12), BF16)
    din("cosf", (64, T))
    din("sinf", (64, T))
    din("cosb", (64, T), BF16)
    din("sinb", (64, T), BF16)
    din("ck1_wT", (128, BLK, 128))
    din("cv1_wT", (128, BLK, 128), BF16)
    din("b1k", (128, 1))
    din("b1v", (128, 1))
    din("ck2", (128, 128))
    din("ck2b", (128, 1))
    din("cv2a", (128, 129), BF16)
    din("brv", (1, 129), BF16)
    din("maskA", (128, TB, NBP))
    din("maskT01", (NBP, T), BF16)
    din("tib", (NBP, T), BF16)
    din("caus01", (128, 128), BF16)
    din("win01", (128, 128), BF16)
    din("identb", (128, 128), BF16)
    din("identf", (128, 128))
    out_dram = nc.dram_tensor("out", [NREP, T, DH], F32, kind="ExternalOutput")

    with tile.TileContext(nc) as tc:
        _emit(nc, tc, dram, out_dram)
    nc.compile()
    return nc


_PROGRAM = None


def _get_program():
    global _PROGRAM
    if _PROGRAM is None:
        _PROGRAM = _build_program()
    return _PROGRAM


def _host_inputs(inputs):
    bf = ml_dtypes.bfloat16
    x = np.asarray(inputs["x"], np.float32)
    q = np.asarray(inputs["q"], np.float32)
    gate_w = np.asarray(inputs["gate_w"], np.float32)
    gate_b = np.asarray(inputs["gate_b"], np.float32)
    block_pos = np.asarray(inputs["block_pos"], np.float32)

    half = DH // 2
    pos = np.arange(T, dtype=np.float32)
    inv = (1.0 / (10000.0 ** (np.arange(half, dtype=np.float32) / half))).astype(np.float32)
    ang = (pos[:, None] * inv[None, :]).astype(np.float32)
    cosf = np.cos(ang.astype(np.float64)).astype(np.float32).T.copy()
    sinf = np.sin(ang.astype(np.float64)).astype(np.float32).T.copy()

    t_idx = np.arange(T)
    live = (t_idx[:, None] >= STARTS[None, :]).astype(np.float32)  # (T, NB)
    maskA = np.concatenate([live, np.zeros((T, 1), np.float32)], 1)  # (T, 64)
    maskA = maskA.reshape(TB, 128, NBP).transpose(1, 0, 2).copy()  # (128, TB, 64)
    maskT01 = np.concatenate([live.T, np.zeros((1, T), np.float32)], 0)  # (64, T)
    tib = ((t_idx[None, :] >= STARTS[:, None])
           & (t_idx[None, :] < STARTS[:, None] + BLK)).astype(np.float32)
    tib = np.concatenate([tib, np.zeros((1, T), np.float32)], 0)
    loc = np.arange(128)
    caus01 = (loc[None, :] >= loc[:, None]).astype(bf)
    win01 = (loc[None, :] < loc[:, None]).astype(bf)
    identf = np.eye(128, dtype=np.float32)

    ws = {k: np.asarray(inputs[k], np.float32) for k in
          ("wk_cmp", "wv_cmp", "wk_slc", "wv_slc", "wk_win", "wv_win")}
    ck1_w = np.asarray(inputs["ck1_w"], np.float32)
    cv1_w = np.asarray(inputs["cv1_w"], np.float32)
    bp_flat = block_pos.reshape(-1)
    b1k = (np.asarray(inputs["ck1_b"], np.float32) + ck1_w @ bp_flat).reshape(128, 1)
    b1v = (np.asarray(inputs["cv1_b"], np.float32) + cv1_w @ bp_flat).reshape(128, 1)
    ck1_wT = ck1_w.reshape(128, BLK, 128).transpose(2, 1, 0).copy()
    cv1_wT = cv1_w.reshape(128, BLK, 128).transpose(2, 1, 0).astype(bf)
    ck2 = np.asarray(inputs["ck2_w"], np.float32).T.copy()
    ck2b = np.asarray(inputs["ck2_b"], np.float32).reshape(128, 1)
    cv2a = np.concatenate([np.asarray(inputs["cv2_w"], np.float32).T,
                           np.zeros((128, 1), np.float32)], 1).astype(bf)
    brv = np.concatenate([np.asarray(inputs["cv2_b"], np.float32),
                          [1.0]]).astype(np.float32).reshape(1, 129).astype(bf)

    def part_major(w):
        # (dout=128, DM) weight -> lhsT layout (128p=dm_chunk, MO, dout)
        return np.ascontiguousarray(w.T.reshape(MO, 128, -1).transpose(1, 0, 2))

    in_maps = []
    for core in range(NCORES):
        b, kv = divmod(core, NKV)
        heads = [g * NKV + kv for g in range(NREP)]
        xT = np.ascontiguousarray(x[b].T.reshape(MO, 128, T).transpose(1, 0, 2))
        xTq = np.ascontiguousarray(
            xT.reshape(128, MO, 8, 128).transpose(2, 0, 1, 3))
        qh = q[b, heads] * SCALE                       # (4, T, DH)
        qT = np.ascontiguousarray(qh.transpose(2, 0, 1))  # (128, 4, T)
        wTl = {k: part_major(w[kv * DH:(kv + 1) * DH]) for k, w in ws.items()}
        wTb = np.stack([wTl["wk_win"], wTl["wv_win"],
                        wTl["wk_slc"], wTl["wv_slc"]]).astype(bf)
        cols = [h * 3 + j for h in heads for j in range(3)]
        gw = np.ascontiguousarray(
            gate_w[cols].T.reshape(MO, 128, 12).transpose(1, 0, 2)).astype(bf)
        gbr = gate_b[cols].reshape(1, 12).astype(bf)
        in_maps.append({
            "xTq": xTq, "xTb": xT.astype(bf),
            "qT": qT, "qTb": qT.astype(bf),
            "wcmp": wTl["wk_cmp"], "wvcmp": wTl["wv_cmp"].astype(bf),
            "wTb": wTb, "gw": gw, "gbr": gbr,
            "cosf": cosf, "sinf": sinf,
            "cosb": cosf.astype(bf), "sinb": sinf.astype(bf),
            "ck1_wT": ck1_wT, "cv1_wT": cv1_wT, "b1k": b1k, "b1v": b1v,
            "ck2": ck2, "ck2b": ck2b, "cv2a": cv2a, "brv": brv,
            "maskA": maskA, "maskT01": maskT01.astype(bf),
            "tib": tib.astype(bf),
            "caus01": caus01, "win01": win01,
            "identb": identf.astype(bf), "identf": identf,
        })
    return in_maps


def kernel(**inputs) -> np.ndarray:
    nc = _get_program()
    in_maps = _host_inputs(inputs)
    res = run_bass_kernel_spmd(nc, in_maps, list(range(NCORES)))
    out = np.empty((B, NQ, T, DH), np.float32)
    for core in range(NCORES):
        b, kv = divmod(core, NKV)
        oc = res.results[core]["out"]
        for g in range(NREP):
            out[b, g * NKV + kv] = oc[g]
    return out


if __name__ == "__main__":
    _get_program()
    print("program built + compiled OK")


# revision 70
# speedup vs baseline: 1.0140x; 1.0140x over previous
"""NativeSparseAttention Trainium2 kernel (8-core SPMD), v3.

Sharding: core c handles (b, kv) = (c // 4, c % 4); all three attention
branches, the gate/compress MLPs, and the k/v projections for that
(batch, kv-head) pair are fully independent across cores.

Numerics (same plan as the validated baseline):
  - branch-1 chain (k_cmp projection, compress MLP, branch-1 scores,
    softmax for p_grp, top-16 selection) in fp32; exp via a degree-6
    polynomial P(s) ~ exp(s/2) squared (rel err ~7e-7; the ACT LUT exp
    is only ~1e-5 and block selection needs ~1e-6 to keep the fp32
    reference's top-k ordering).
  - branches 2/3 in bf16 on the PE with fp32 PSUM accumulation;
    branch outputs held in bf16, combined with fp32 accumulation.
  - softmax skips max-subtraction (live score range is small at this
    model's scale) and normalizes after the PV matmul via an appended
    ones-column in V (row-sum lands in output column 128).

Scheduling: slc/win projections run first; branch-3 and branch-2
score/exp work is emitted between chunks of the fp32 k_cmp projection
so ACT/DVE overlap the PE-heavy phase; bf16 elementwise sits on DVE
(2x mode); every DMA is contiguous per partition and issued from the
SP queue (gpsimd-issued DMAs consume Pool engine time).
"""

import sys
import os

USE_LUT_EXP = os.environ.get("NSA_LUT", "0") == "1"
# bitmask of "safe construct" fallbacks for HW-compiler bisection
SAFE = int(os.environ.get("NSA_SAFE", "0"))
S_PGRP, S_COMB, S_BCAST, S_TRANS, S_ISGT, S_GPCP = 1, 2, 4, 8, 16, 32

for _p in ("/opt/trn_rl_repo", "/root/.axon_site/_ro/trn_rl_repo"):
    if _p not in sys.path:
        sys.path.append(_p)

import numpy as np
import ml_dtypes

import concourse.bass as bass
import concourse.mybir as mybir
import concourse.tile as tile
from concourse import bacc
from concourse.bass_utils import run_bass_kernel_spmd

AF = mybir.ActivationFunctionType
ALU = mybir.AluOpType
F32 = mybir.dt.float32
BF16 = mybir.dt.bfloat16

B, T, DM = 2, 1024, 2048
NQ, NKV, DH = 16, 4, 128
BLK, STRIDE, TOPN, WIN = 32, 16, 16, 512
NREP = NQ // NKV
NB = 63
NBP = 64                    # padded block count (col 63 is dead)
MO = DM // 128
TB = T // 128
TC = T // 512
SCALE = DH ** -0.5
STARTS = np.minimum(np.arange(NB) * STRIDE, T - 1)
NCORES = 8

# degree-6 fit of exp(s/2) on |s| <= 1.3; exp(s) = P(s)^2, rel err ~7e-7
_xs = np.cos(np.pi * (np.arange(8000) + 0.5) / 8000) * 1.3
_V = np.vander(_xs, 7, increasing=True)
EXPC = [float(v) for v in np.linalg.lstsq(_V, np.exp(_xs / 2), rcond=None)[0]]


def _emit(nc, tc, d, out_dram):
    def ap(name):
        return d[name].ap()

    from contextlib import ExitStack
    stk = ExitStack()
    consts = stk.enter_context(tc.tile_pool(name="consts", bufs=1))
    pers = stk.enter_context(tc.tile_pool(name="pers", bufs=1))
    pp = stk.enter_context(tc.tile_pool(name="pp", bufs=2, space="PSUM"))
    psS = stk.enter_context(tc.tile_pool(name="psS", bufs=2, space="PSUM"))
    psV = stk.enter_context(tc.tile_pool(name="psV", bufs=2, space="PSUM"))
    psA = stk.enter_context(tc.tile_pool(name="psA", bufs=1, space="PSUM"))

    # ---------------- persistent tiles ----------------
    qb_sb = pers.tile([128, NREP, T], BF16, tag="qb")
    kslcT = pers.tile([128, T], BF16, tag="kslcT")
    kwinT = pers.tile([128, T], BF16, tag="kwinT")
    vslc = pers.tile([128, TB, 129], BF16, tag="vslc")
    vwin = pers.tile([128, TB, 129], BF16, tag="vwin")
    nc.vector.memset(vslc[:, :, 128:129], 1.0)
    nc.vector.memset(vwin[:, :, 128:129], 1.0)
    kcmpT = pers.tile([128, T], F32, tag="kcmpT")
    vcmpT = pers.tile([128, T], BF16, tag="vcmpT")
    gates = pers.tile([128, TB, 12], F32, tag="gates")
    ksumT = pers.tile([128, NBP], F32, tag="ksumT")
    ksum_bf = pers.tile([128, NBP], BF16, tag="ksumbf")
    vsuma_bf = pers.tile([NBP, 129], BF16, tag="vsumabf")
    o_win = pers.tile([128, TB, NREP, 129], BF16, tag="owin")
    h_k = pers.tile([128, NBP], F32, tag="hk")
    h_v = pers.tile([128, NBP], BF16, tag="hv")
    qpool = stk.enter_context(tc.tile_pool(name="qpool", bufs=1))
    # e2 lives through B..G; layout groups the head dim so one m01
    # multiply covers all four heads via a broadcast middle dim
    e2a = stk.enter_context(tc.tile_pool(name="e2a", bufs=1))
    e2 = e2a.tile([128, 12, NREP, 512], BF16, tag="e2")

    # ---------------- consts (sync DMA queue, ordered by first use) -------
    identb = consts.tile([128, 128], BF16, tag="identb")
    identf = consts.tile([128, 128], F32, tag="identf")
    caus01 = consts.tile([128, 128], BF16, tag="caus01")
    win01 = consts.tile([128, 128], BF16, tag="win01")
    tib_sb = consts.tile([NBP, T], BF16, tag="tib")
    maskA = consts.tile([128, TB, NBP], F32, tag="maskA")
    maskT01 = consts.tile([NBP, T], BF16, tag="maskT01")
    gw_sb = consts.tile([128, MO, 12], BF16, tag="gw")
    gbr = consts.tile([1, 12], BF16, tag="gbr")
    onesb = consts.tile([1, 128], BF16, tag="onesb")
    brv = consts.tile([1, 129], BF16, tag="brv")
    b1k = consts.tile([128, 1], F32, tag="b1k")
    b1v = consts.tile([128, 1], F32, tag="b1v")
    ck2_sb = consts.tile([128, 128], F32, tag="ck2")
    ck2b = consts.tile([128, 1], F32, tag="ck2b")
    cv2a = consts.tile([128, 129], BF16, tag="cv2a")
    c = EXPC
    c0b = consts.tile([128, 1], F32, tag="c0b")
    c4b = consts.tile([128, 1], F32, tag="c4b")
    nc.vector.memset(c0b[:], c[0])
    nc.vector.memset(c4b[:], c[4])
    nc.vector.memset(onesb[:], 1.0)

    # ================= stage A: slc/win projections =================
    epool = stk.enter_context(tc.tile_pool(name="epool", bufs=1))
    stkAD = ExitStack()
    projp = stkAD.enter_context(tc.tile_pool(name="projp", bufs=1))
    trig = stkAD.enter_context(tc.tile_pool(name="trig", bufs=1))
    wstrm = stkAD.enter_context(tc.tile_pool(name="wstrm", bufs=2))
    ev = stkAD.enter_context(tc.tile_pool(name="ev", bufs=2))

    xb_sb = projp.tile([128, MO, T], BF16, tag="xb")
    w0_sb = wstrm.tile([128, MO, 128], BF16, tag="wcur")
    nc.sync.dma_start(w0_sb[:], ap("wTb")[0])
    nc.sync.dma_start(xb_sb[:, :, 0:512], ap("xTb")[:, :, 0:512])
    cosb = trig.tile([64, T], BF16, tag="cosb")
    sinb = trig.tile([64, T], BF16, tag="sinb")
    cosf = trig.tile([64, T], F32, tag="cosf")
    sinf = trig.tile([64, T], F32, tag="sinf")
    nc.sync.dma_start(cosb[:], ap("cosb"))
    nc.sync.dma_start(sinb[:], ap("sinb"))
    nc.sync.dma_start(xb_sb[:, :, 512:T], ap("xTb")[:, :, 512:T])

    def rope_bf(ps, tck, outT):
        # bf16 rotate-half rope from psum [128, 512] into outT[128, T] slice
        sl = slice(tck * 512, (tck + 1) * 512)
        tlo = ev.tile([64, 512], BF16, tag="tlo")
        thi = ev.tile([64, 512], BF16, tag="thi")
        nc.scalar.copy(tlo[:], ps[0:64, :])
        nc.scalar.copy(thi[:], ps[64:128, :])
        ta = ev.tile([64, 512], BF16, tag="ropa")
        tb_ = ev.tile([64, 512], BF16, tag="ropb")
        cc = cosb[:, sl]
        ss = sinb[:, sl]
        nc.vector.tensor_tensor(ta[:], tlo[:], cc, op=ALU.mult)
        nc.vector.tensor_tensor(tb_[:], thi[:], ss, op=ALU.mult)
        nc.vector.tensor_sub(outT[0:64, sl], ta[:], tb_[:])
        nc.vector.tensor_tensor(ta[:], tlo[:], ss, op=ALU.mult)
        nc.vector.tensor_tensor(tb_[:], thi[:], cc, op=ALU.mult)
        nc.vector.tensor_add(outT[64:128, sl], ta[:], tb_[:])

    def v_evict(ps, tck, vdst):
        if SAFE & S_TRANS:
            tmp = ev.tile([128, 512], F32, tag="vtmpf")
            nc.scalar.copy(tmp[:], ps[:])
            for j in range(4):
                kt = tck * 4 + j
                pst = psA.tile([128, 129], F32, tag="X")
                nc.tensor.transpose(pst[:, 0:128], tmp[:, j * 128:(j + 1) * 128],
                                    identf[:])
                nc.vector.tensor_copy(vdst[:, kt, 0:128], pst[:, 0:128])
            return
        tmp = ev.tile([128, 512], BF16, tag="vtmp")
        nc.scalar.copy(tmp[:], ps[:])
        for j in range(4):
            kt = tck * 4 + j
            pst = psA.tile([128, 128], BF16, tag="Xb")
            nc.tensor.transpose(pst[:], tmp[:, j * 128:(j + 1) * 128], identb[:])
            nc.vector.tensor_copy(vdst[:, kt, 0:128], pst[:])

    for wi, (kind, dst) in enumerate((("k", kwinT), ("v", vwin),
                                      ("k", kslcT), ("v", vslc))):
        if wi == 0:
            w_wi = w0_sb
        else:
            w_wi = wstrm.tile([128, MO, 128], BF16, tag="wcur")
            nc.sync.dma_start(w_wi[:], ap("wTb")[wi])
        if wi == 1:
            nc.sync.dma_start(qb_sb[:], ap("qTb"))
        for tck in range(TC):
            ps = pp.tile([128, 512], F32, tag="P")
            for mo in range(MO):
                nc.tensor.matmul(ps[:], w_wi[:, mo, :],
                                 xb_sb[:, mo, tck * 512:(tck + 1) * 512],
                                 start=(mo == 0), stop=(mo == MO - 1))
            if kind == "k":
                rope_bf(ps, tck, dst)
            else:
                v_evict(ps, tck, dst)

    for t_, n_ in ((caus01, "caus01"), (win01, "win01"), (cosf, "cosf"),
                   (sinf, "sinf"), (gw_sb, "gw"), (gbr, "gbr"),
                   (identb, "identb"), (identf, "identf")):
        nc.sync.dma_start(t_[:], ap(n_))

    # gates: [t, 12] per tb
    for tb in range(TB):
        ps = psA.tile([128, 129], F32, tag="X")
        for mo in range(MO):
            nc.tensor.matmul(ps[:, 0:12], xb_sb[:, mo, tb * 128:(tb + 1) * 128],
                             gw_sb[:, mo, :], start=(mo == 0), stop=False)
        nc.tensor.matmul(ps[:, 0:12], onesb[:], gbr[:], start=False, stop=True)
        nc.scalar.activation(gates[:, tb, :], ps[:, 0:12], AF.Sigmoid)

    # ================= stage B: fp32 k_cmp proj + branch 3 + b2 sc ========
    xstrm = stkAD.enter_context(tc.tile_pool(name="xstrm", bufs=2))
    wcp = projp.tile([128, MO, 128], F32, tag="wcmp")
    nc.sync.dma_start(wcp[:], ap("wcmp"))

    def rope_f32(ps, ch, eng):
        # gpsimd cannot read PSUM: stage the two halves into SBUF via ACT
        sl = slice(ch * 128, (ch + 1) * 128)
        cc = cosf[:, sl]
        ss = sinf[:, sl]
        flo = ev.tile([64, 128], F32, tag="flo")
        fhi = ev.tile([64, 128], F32, tag="fhi")
        nc.scalar.copy(flo[:], ps[0:64, :])
        nc.scalar.copy(fhi[:], ps[64:128, :])
        ta = ev.tile([64, 128], F32, tag="fra")
        tb_ = ev.tile([64, 128], F32, tag="frb")
        tc_ = ev.tile([64, 128], F32, tag="frc")
        td_ = ev.tile([64, 128], F32, tag="frd")
        eng.tensor_tensor(ta[:], flo[:], cc, op=ALU.mult)
        eng.tensor_tensor(tb_[:], fhi[:], ss, op=ALU.mult)
        eng.tensor_sub(kcmpT[0:64, sl], ta[:], tb_[:])
        eng.tensor_tensor(tc_[:], flo[:], ss, op=ALU.mult)
        eng.tensor_tensor(td_[:], fhi[:], cc, op=ALU.mult)
        eng.tensor_add(kcmpT[64:128, sl], tc_[:], td_[:])

    def emit_b3(g):
        for i in range(TB):
            sl = slice(i * 128, (i + 1) * 128)
            kts = list(range(max(0, i - 4), i + 1))
            groups = [kts[j:j + 4] for j in range(0, len(kts), 4)]
            e3 = {}
            for grp in groups:
                ps = psS.tile([128, 512], F32, tag="S")
                for j, kt in enumerate(grp):
                    nc.tensor.matmul(ps[:, j * 128:(j + 1) * 128],
                                     kwinT[:, kt * 128:(kt + 1) * 128],
                                     qb_sb[:, g, sl], start=True, stop=True)
                et = epool.tile([128, 4, 128], BF16, tag=f"e3g{grp[0] % 3}")
                nc.scalar.activation(
                    et[:, 0:len(grp), :],
                    ps[:, 0:len(grp) * 128].rearrange("p (a b) -> p a b", b=128),
                    AF.Exp)
                for j, kt in enumerate(grp):
                    if kt == i:
                        nc.vector.tensor_tensor(et[:, j, :], et[:, j, :],
                                                caus01[:], op=ALU.mult)
                    elif kt == i - 4:
                        nc.vector.tensor_tensor(et[:, j, :], et[:, j, :],
                                                win01[:], op=ALU.mult)
                    e3[kt] = et[:, j, :]
            psv = psV.tile([128, 129], F32, tag="V")
            for kt in kts:
                nc.tensor.matmul(psv[:], e3[kt], vwin[:, kt, :],
                                 start=(kt == kts[0]), stop=(kt == kts[-1]))
            nc.scalar.copy(o_win[:, i, g, :], psv[:])

    def emit_b2sc(g):
        # branch-2 scores + exp for head-group g into e2[:, j, g, :]
        for tck in range(TC):
            for kt in range(4 * tck + 4):
                j = kt if tck == 0 else 4 + kt
                qs = max(0, kt * 128 - tck * 512)
                ps = psS.tile([128, 512], F32, tag="S")
                nc.tensor.matmul(ps[:, qs:512], kslcT[:, kt * 128:(kt + 1) * 128],
                                 qb_sb[:, g, tck * 512 + qs:(tck + 1) * 512],
                                 start=True, stop=True)
                nc.scalar.activation(e2[:, j, g, qs:512], ps[:, qs:512], AF.Exp)

    for qtr in range(4):
        for hf in range(2):
            ch = qtr * 2 + hf
            ps = pp.tile([128, 512], F32, tag="P")
            for moh in range(2):
                xq = xstrm.tile([128, 8, 128], F32, tag="xq")
                nc.sync.dma_start(xq[:], ap("xTq")[ch, :, moh * 8:(moh + 1) * 8])
                for mo in range(8):
                    nc.tensor.matmul(ps[:, 0:128], wcp[:, moh * 8 + mo, :],
                                     xq[:, mo, :],
                                     start=(moh == 0 and mo == 0),
                                     stop=(moh == 1 and mo == 7))
            rope_f32(ps[:, 0:128], ch,
                     nc.vector if qtr % 2 == 0 else nc.gpsimd)
        emit_b3(qtr)
        if qtr >= 1:
            emit_b2sc(qtr - 1)

    # ================= stage C: v_cmp projection =================
    q_sb = qpool.tile([128, NREP, T], F32, tag="q")
    nc.sync.dma_start(q_sb[:], ap("qT"))
    wvc = wstrm.tile([128, MO, 128], BF16, tag="wcur")
    nc.sync.dma_start(wvc[:], ap("wvcmp"))
    for tck in range(TC):
        ps = pp.tile([128, 512], F32, tag="P")
        for mo in range(MO):
            nc.tensor.matmul(ps[:], wvc[:, mo, :],
                             xb_sb[:, mo, tck * 512:(tck + 1) * 512],
                             start=(mo == 0), stop=(mo == MO - 1))
        nc.scalar.copy(vcmpT[:, tck * 512:(tck + 1) * 512], ps[:])
    emit_b2sc(3)

    # ================= stage D: compress MLPs =================
    for t_, n_ in ((maskA, "maskA"), (maskT01, "maskT01"), (tib_sb, "tib"),
                   (b1k, "b1k"), (b1v, "b1v"), (ck2_sb, "ck2"),
                   (ck2b, "ck2b"), (cv2a, "cv2a"), (brv, "brv")):
        nc.sync.dma_start(t_[:], ap(n_))
    for wn, srcT, bias1, h, dt_ in (("ck1_wT", kcmpT, b1k, h_k, F32),
                                    ("cv1_wT", vcmpT, b1v, h_v, BF16)):
        ps = pp.tile([128, 512], F32, tag="P")
        for cg in range(2):
            w1c = wstrm.tile([128, 16, 128], dt_, tag=f"w1c{dt_}", bufs=1)
            nc.sync.dma_start(w1c[:], ap(wn)[:, cg * 16:(cg + 1) * 16, :])
            for cc in range(16):
                c_ = cg * 16 + cc
                rhs = srcT[:, c_:c_ + 16 * (NB - 1) + 1:16]
                nc.tensor.matmul(ps[:, 0:NB], w1c[:, cc, :], rhs,
                                 start=(c_ == 0), stop=(c_ == BLK - 1))
        nc.vector.memset(h[:, NB:NBP], 0.0)
        nc.scalar.activation(h[:, 0:NB], ps[:, 0:NB], AF.Gelu, bias=bias1[:])

    ps = pp.tile([128, 512], F32, tag="P")
    nc.tensor.matmul(ps[:, 0:NBP], ck2_sb[:], h_k[:], start=True, stop=True)
    nc.scalar.activation(ksumT[:], ps[:, 0:NBP], AF.Identity, bias=ck2b[:])
    nc.vector.tensor_copy(ksum_bf[:], ksumT[:])

    ps = psA.tile([128, 129], F32, tag="X")
    nc.tensor.matmul(ps[0:NBP, :], h_v[:], cv2a[:], start=True, stop=False)
    nc.tensor.matmul(ps[0:NBP, :], onesb[:, 0:NBP], brv[:], start=False, stop=True)
    nc.vector.tensor_copy(vsuma_bf[:], ps[0:NBP, :])

    # ================= stage E: branch 1 (poly softmax) =================
    stkAD.close()
    pers2 = stk.enter_context(tc.tile_pool(name="pers2", bufs=1))
    pgrp = pers2.tile([128, TB, NBP], F32, tag="pgrp")
    m01 = pers2.tile([128, TB, T], BF16, tag="m01")
    o_cmp = pers2.tile([128, TB, NREP, 129], BF16, tag="ocmp")
    o_slc = pers2.tile([128, TB, NREP, 129], BF16, tag="oslc")
    selT = pers2.tile([NBP, T], BF16, tag="selT")
    polyp = stk.enter_context(tc.tile_pool(name="polyp", bufs=2))

    def emit_b1(g):
        pss = psS.tile([128, 512], F32, tag="S")
        for tb in range(TB):
            nc.tensor.matmul(pss[:, tb * 64:(tb + 1) * 64],
                             q_sb[:, g, tb * 128:(tb + 1) * 128],
                             ksumT[:], start=True, stop=True)
        eA = polyp.tile([128, TB, NBP], F32, tag="eA")
        S = polyp.tile([128, TB, 1], F32, tag="pS")
        r = polyp.tile([128, TB, 1], F32, tag="pr")
        halves = ((nc.vector, slice(0, 5)), (nc.gpsimd, slice(5, TB)))

        def tt(out, in0, in1, op=ALU.mult):
            for eng, hs in halves:
                eng.tensor_tensor(out[:, hs, :], in0[:, hs, :],
                                  in1[:, hs, :] if in1.shape[1] == TB else in1,
                                  op=op)

        if USE_LUT_EXP:
            s3 = pss[:].rearrange("p (a b) -> p a b", b=NBP)
            nc.scalar.activation(eA[:], s3, AF.Exp)
            tt(eA, eA, maskA)
        else:
            # P(s) = (c0+c1 s) + w(c2+c3 s) + v(c4+c5 s + c6 w); exp(s) = P^2
            sA = polyp.tile([128, TB, NBP], F32, tag="sA")
            nc.scalar.copy(sA[:].rearrange("p a b -> p (a b)"), pss[:])
            w_ = polyp.tile([128, TB, NBP], F32, tag="w")
            v_ = polyp.tile([128, TB, NBP], F32, tag="v")
            t1 = polyp.tile([128, TB, NBP], F32, tag="t1")
            t2 = polyp.tile([128, TB, NBP], F32, tag="t2")
            t3 = polyp.tile([128, TB, NBP], F32, tag="t3")
            nc.vector.tensor_tensor(w_[:], sA[:], sA[:], op=ALU.mult)
            nc.scalar.activation(t1[:], sA[:], AF.Identity, bias=c0b[:],
                                 scale=c[1])
            nc.vector.tensor_scalar(t2[:], sA[:], c[3], c[2],
                                    op0=ALU.mult, op1=ALU.add)
            nc.scalar.activation(t3[:], sA[:], AF.Identity, bias=c4b[:],
                                 scale=c[5])
            nc.vector.scalar_tensor_tensor(t3[:], w_[:], c[6], t3[:],
                                           op0=ALU.mult, op1=ALU.add)
            nc.gpsimd.tensor_tensor(v_[:], w_[:], w_[:], op=ALU.mult)
            tt(t2, w_, t2)
            tt(t1, t1, t2, op=ALU.add)
            tt(t3, v_, t3)
            tt(t1, t1, t3, op=ALU.add)
            tt(t1, t1, maskA)
            tt(eA, t1, t1)
        nc.vector.reduce_sum(S[:], eA[:], axis=mybir.AxisListType.X)
        nc.vector.reciprocal(r[:], S[:])
        if SAFE & S_PGRP:
            rb = r[:].to_broadcast([128, TB, NBP])
            nc.vector.tensor_tensor(eA[:, 0:5, :], eA[:, 0:5, :], rb[:, 0:5, :],
                                    op=ALU.mult)
            nc.gpsimd.tensor_tensor(eA[:, 5:TB, :], eA[:, 5:TB, :],
                                    rb[:, 5:TB, :], op=ALU.mult)
            if g == 0:
                nc.vector.tensor_copy(pgrp[:], eA[:])
            else:
                nc.vector.tensor_add(pgrp[:], pgrp[:], eA[:])
        else:
            # pgrp[:, tb, :] (+)= eA[:, tb, :] * r[tb]  (per-partition scalar)
            for tb in range(TB):
                if g == 0:
                    nc.vector.tensor_scalar(pgrp[:, tb, :], eA[:, tb, :],
                                            r[:, tb, :], None, op0=ALU.mult)
                else:
                    nc.vector.scalar_tensor_tensor(pgrp[:, tb, :], eA[:, tb, :],
                                                   r[:, tb, :], pgrp[:, tb, :],
                                                   op0=ALU.mult, op1=ALU.add)

        # branch-1 output path
        eTt = polyp.tile([NBP, T], BF16, tag="eT")
        for tck in range(TC):
            sl = slice(tck * 512, (tck + 1) * 512)
            ps = pp.tile([128, 512], F32, tag="P")
            nc.tensor.matmul(ps[0:NBP, :], ksum_bf[:], qb_sb[:, g, sl],
                             start=True, stop=True)
            nc.scalar.activation(eTt[:, sl], ps[0:NBP, :], AF.Exp)
            nc.vector.tensor_tensor(eTt[:, sl], eTt[:, sl], maskT01[:, sl],
                                    op=ALU.mult)
        for tb in range(TB):
            psv = psV.tile([128, 129], F32, tag="V")
            nc.tensor.matmul(psv[:], eTt[:, tb * 128:(tb + 1) * 128],
                             vsuma_bf[:], start=True, stop=True)
            nc.scalar.copy(o_cmp[:, tb, g, :], psv[:])

    for g in range(NREP):
        emit_b1(g)

    # ================= stage F: top-16 + coverage mask + e2 masking =======
    fpool = stk.enter_context(tc.tile_pool(name="fpool", bufs=2))
    for tb in range(TB):
        mx = fpool.tile([128, 8], F32, tag="mx")
        sw = fpool.tile([128, NBP], F32, tag="sw")
        nc.vector.max(mx[:], pgrp[:, tb, :])
        nc.vector.match_replace(sw[:], mx[:], pgrp[:, tb, :], 0.0)
        nc.vector.max(mx[:], sw[:])
        nc.vector.match_replace(sw[:], mx[:], sw[:], 0.0)
        if SAFE & S_ISGT:
            nc.vector.tensor_sub(sw[:], pgrp[:, tb, :], sw[:])
            nc.vector.tensor_scalar(sw[:], sw[:], 0.0, None, op0=ALU.is_gt)
        else:
            nc.vector.tensor_tensor(sw[:], pgrp[:, tb, :], sw[:], op=ALU.is_gt)
        pst = psA.tile([128, 129], F32, tag="X")
        nc.tensor.transpose(pst[0:NBP, 0:128], sw[:], identf[:])
        nc.scalar.copy(selT[:, tb * 128:(tb + 1) * 128], pst[0:NBP, 0:128])

    # m01 per key tile sc, immediately mask e2 and run the PV matmuls for
    # query tile i == sc (all earlier key tiles are already masked)
    for sc in range(TB):
        for tck in range((sc * 128) // 512, TC):
            qs = max(0, sc * 128 - tck * 512)
            sl = slice(tck * 512 + qs, (tck + 1) * 512)
            ps = psS.tile([128, 512], F32, tag="S")
            nc.tensor.matmul(ps[:, qs:512], tib_sb[:, sc * 128:(sc + 1) * 128],
                             selT[:, sl], start=True, stop=True)
            nc.vector.tensor_scalar(m01[:, sc, sl], ps[:, qs:512], 0.0, None,
                                    op0=ALU.is_gt)
        nc.vector.tensor_tensor(m01[:, sc, sc * 128:(sc + 1) * 128],
                                m01[:, sc, sc * 128:(sc + 1) * 128],
                                caus01[:], op=ALU.mult)
        # mask e2 for this key tile: one broadcast op covers all four heads
        for tck in range((sc * 128) // 512, TC):
            j = sc if tck == 0 else 4 + sc
            qs = max(0, sc * 128 - tck * 512)
            sl = slice(tck * 512 + qs, (tck + 1) * 512)
            if SAFE & S_BCAST:
                for g_ in range(NREP):
                    eng = nc.vector if g_ % 2 == 0 else nc.gpsimd
                    eng.tensor_tensor(e2[:, j, g_, qs:512],
                                      e2[:, j, g_, qs:512],
                                      m01[:, sc, sl], op=ALU.mult)
            else:
                mb = m01[:, sc:sc + 1, sl].to_broadcast([128, NREP, 512 - qs])
                nc.vector.tensor_tensor(e2[:, j, :, qs:512],
                                        e2[:, j, :, qs:512], mb, op=ALU.mult)
        # branch-2 PV for query tile i == sc (keys kt <= sc all masked now)
        i = sc
        tck = i // 4
        lo = (i - 4 * tck) * 128
        for g in range(NREP):
            psv = psV.tile([128, 129], F32, tag="V")
            for kt in range(i + 1):
                j = kt if tck == 0 else 4 + kt
                nc.tensor.matmul(psv[:], e2[:, j, g, lo:lo + 128],
                                 vslc[:, kt, :], start=(kt == 0),
                                 stop=(kt == i))
            nc.scalar.copy(o_slc[:, i, g, :], psv[:])

    # ================= stage G: combine =================
    for g in range(NREP):
        # normalize + gate + combine:  acc = sum_j gate_j/Z_j * o_j
        w0 = fpool.tile([128, TB, 1], F32, tag="w0")
        w1 = fpool.tile([128, TB, 1], F32, tag="w1")
        w2 = fpool.tile([128, TB, 1], F32, tag="w2")
        for wj, o_un, jj in ((w0, o_cmp, 0), (w1, o_slc, 1), (w2, o_win, 2)):
            nc.vector.reciprocal(wj[:], o_un[:, :, g, 128:129])
            nc.vector.tensor_tensor(wj[:], wj[:],
                                    gates[:, :, 3 * g + jj:3 * g + jj + 1],
                                    op=ALU.mult)
        accf = fpool.tile([128, TB, 128], F32, tag="accf")
        if SAFE & S_COMB:
            tmp = fpool.tile([128, TB, 128], F32, tag="tmp", bufs=1)
            tmp2 = fpool.tile([128, TB, 128], F32, tag="tmp2", bufs=1)
            nc.vector.tensor_tensor(accf[:], o_cmp[:, :, g, 0:128],
                                    w0[:].to_broadcast([128, TB, 128]),
                                    op=ALU.mult)
            nc.gpsimd.tensor_tensor(tmp[:], o_slc[:, :, g, 0:128],
                                    w1[:].to_broadcast([128, TB, 128]),
                                    op=ALU.mult)
            nc.vector.tensor_tensor(tmp2[:], o_win[:, :, g, 0:128],
                                    w2[:].to_broadcast([128, TB, 128]),
                                    op=ALU.mult)
            nc.vector.tensor_add(accf[:], accf[:], tmp[:])
            nc.gpsimd.tensor_add(accf[:], accf[:], tmp2[:])
        else:
            accb = fpool.tile([128, TB, 128], BF16, tag="accb")
            accb2 = fpool.tile([128, TB, 128], BF16, tag="accb2")
            for tb in range(TB):
                nc.scalar.activation(accb[:, tb, :], o_cmp[:, tb, g, 0:128],
                                     AF.Identity, scale=w0[:, tb, :])
                nc.vector.tensor_scalar(accb2[:, tb, :], o_slc[:, tb, g, 0:128],
                                        w1[:, tb, :], None, op0=ALU.mult)
            nc.vector.tensor_add(accb[:], accb[:], accb2[:])
            for tb in range(TB):
                nc.vector.scalar_tensor_tensor(accf[:, tb, :],
                                               o_win[:, tb, g, 0:128],
                                               w2[:, tb, :], accb[:, tb, :],
                                               op0=ALU.mult, op1=ALU.add)
        nc.sync.dma_start(out_dram.ap()[g].rearrange("(tb p) d -> p tb d", p=128),
                          accf[:])

    stk.close()


def _build_program():
    nc = bacc.Bacc("TRN2", target_bir_lowering=False, debug=False,
                   num_devices=NCORES)
    dram = {}

    def din(name, shape, dtype=F32):
        dram[name] = nc.dram_tensor(name, list(shape), dtype, kind="ExternalInput")

    din("xTq", (8, 128, MO, 128))
    din("xTb", (128, MO, T), BF16)
    din("qT", (128, NREP, T))
    din("qTb", (128, NREP, T), BF16)
    din("wcmp", (128, MO, 128))
    din("wvcmp", (128, MO, 128), BF16)
    din("wTb", (4, 128, MO, 128), BF16)
    din("gw", (128, MO, 12), BF16)
    din("gbr", (1, 12), BF16)
    din("cosf", (64, T))
    din("sinf", (64, T))
    din("cosb", (64, T), BF16)
    din("sinb", (64, T), BF16)
    din("ck1_wT", (128, BLK, 128))
    din("cv1_wT", (128, BLK, 128), BF16)
    din("b1k", (128, 1))
    din("b1v", (128, 1))
    din("ck2", (128, 128))
    din("ck2b", (128, 1))
    din("cv2a", (128, 129), BF16)
    din("brv", (1, 129), BF16)
    din("maskA", (128, TB, NBP))
    din("maskT01", (NBP, T), BF16)
    din("tib", (NBP, T), BF16)
    din("caus01", (128, 128), BF16)
    din("win01", (128, 128), BF16)
    din("identb", (128, 128), BF16)
    din("identf", (128, 128))
    out_dram = nc.dram_tensor("out", [NREP, T, DH], F32, kind="ExternalOutput")

    with tile.TileContext(nc) as tc:
        _emit(nc, tc, dram, out_dram)
    nc.compile()
    return nc


_PROGRAM = None


def _get_program():
    global _PROGRAM
    if _PROGRAM is None:
        _PROGRAM = _build_program()
    return _PROGRAM


def _host_inputs(inputs):
    bf = ml_dtypes.bfloat16
    x = np.asarray(inputs["x"], np.float32)
    q = np.asarray(inputs["q"], np.float32)
    gate_w = np.asarray(inputs["gate_w"], np.float32)
    gate_b = np.asarray(inputs["gate_b"], np.float32)
    block_pos = np.asarray(inputs["block_pos"], np.float32)

    half = DH // 2
    pos = np.arange(T, dtype=np.float32)
    inv = (1.0 / (10000.0 ** (np.arange(half, dtype=np.float32) / half))).astype(np.float32)
    ang = (pos[:, None] * inv[None, :]).astype(np.float32)
    cosf = np.cos(ang.astype(np.float64)).astype(np.float32).T.copy()
    sinf = np.sin(ang.astype(np.float64)).astype(np.float32).T.copy()

    t_idx = np.arange(T)
    live = (t_idx[:, None] >= STARTS[None, :]).astype(np.float32)  # (T, NB)
    maskA = np.concatenate([live, np.zeros((T, 1), np.float32)], 1)  # (T, 64)
    maskA = maskA.reshape(TB, 128, NBP).transpose(1, 0, 2).copy()  # (128, TB, 64)
    maskT01 = np.concatenate([live.T, np.zeros((1, T), np.float32)], 0)  # (64, T)
    tib = ((t_idx[None, :] >= STARTS[:, None])
           & (t_idx[None, :] < STARTS[:, None] + BLK)).astype(np.float32)
    tib = np.concatenate([tib, np.zeros((1, T), np.float32)], 0)
    loc = np.arange(128)
    caus01 = (loc[None, :] >= loc[:, None]).astype(bf)
    win01 = (loc[None, :] < loc[:, None]).astype(bf)
    identf = np.eye(128, dtype=np.float32)

    ws = {k: np.asarray(inputs[k], np.float32) for k in
          ("wk_cmp", "wv_cmp", "wk_slc", "wv_slc", "wk_win", "wv_win")}
    ck1_w = np.asarray(inputs["ck1_w"], np.float32)
    cv1_w = np.asarray(inputs["cv1_w"], np.float32)
    bp_flat = block_pos.reshape(-1)
    b1k = (np.asarray(inputs["ck1_b"], np.float32) + ck1_w @ bp_flat).reshape(128, 1)
    b1v = (np.asarray(inputs["cv1_b"], np.float32) + cv1_w @ bp_flat).reshape(128, 1)
    ck1_wT = ck1_w.reshape(128, BLK, 128).transpose(2, 1, 0).copy()
    cv1_wT = cv1_w.reshape(128, BLK, 128).transpose(2, 1, 0).astype(bf)
    ck2 = np.asarray(inputs["ck2_w"], np.float32).T.copy()
    ck2b = np.asarray(inputs["ck2_b"], np.float32).reshape(128, 1)
    cv2a = np.concatenate([np.asarray(inputs["cv2_w"], np.float32).T,
                           np.zeros((128, 1), np.float32)], 1).astype(bf)
    brv = np.concatenate([np.asarray(inputs["cv2_b"], np.float32),
                          [1.0]]).astype(np.float32).reshape(1, 129).astype(bf)

    def part_major(w):
        # (dout=128, DM) weight -> lhsT layout (128p=dm_chunk, MO, dout)
        return np.ascontiguousarray(w.T.reshape(MO, 128, -1).transpose(1, 0, 2))

    in_maps = []
    for core in range(NCORES):
        b, kv = divmod(core, NKV)
        heads = [g * NKV + kv for g in range(NREP)]
        xT = np.ascontiguousarray(x[b].T.reshape(MO, 128, T).transpose(1, 0, 2))
        xTq = np.ascontiguousarray(
            xT.reshape(128, MO, 8, 128).transpose(2, 0, 1, 3))
        qh = q[b, heads] * SCALE                       # (4, T, DH)
        qT = np.ascontiguousarray(qh.transpose(2, 0, 1))  # (128, 4, T)
        wTl = {k: part_major(w[kv * DH:(kv + 1) * DH]) for k, w in ws.items()}
        wTb = np.stack([wTl["wk_win"], wTl["wv_win"],
                        wTl["wk_slc"], wTl["wv_slc"]]).astype(bf)
        cols = [h * 3 + j for h in heads for j in range(3)]
        gw = np.ascontiguousarray(
            gate_w[cols].T.reshape(MO, 128, 12).transpose(1, 0, 2)).astype(bf)
        gbr = gate_b[cols].reshape(1, 12).astype(bf)
        in_maps.append({
            "xTq": xTq, "xTb": xT.astype(bf),
            "qT": qT, "qTb": qT.astype(bf),
            "wcmp": wTl["wk_cmp"], "wvcmp": wTl["wv_cmp"].astype(bf),
            "wTb": wTb, "gw": gw, "gbr": gbr,
            "cosf": cosf, "sinf": sinf,
            "cosb": cosf.astype(bf), "sinb": sinf.astype(bf),
            "ck1_wT": ck1_wT, "cv1_wT": cv1_wT, "b1k": b1k, "b1v": b1v,
            "ck2": ck2, "ck2b": ck2b, "cv2a": cv2a, "brv": brv,
            "maskA": maskA, "maskT01": maskT01.astype(bf),
            "tib": tib.astype(bf),
            "caus01": caus01, "win01": win01,
            "identb": identf.astype(bf), "identf": identf,
        })
    return in_maps


def kernel(**inputs) -> np.ndarray:
    nc = _get_program()
    in_maps = _host_inputs(inputs)
    res = run_bass_kernel_spmd(nc, in_maps, list(range(NCORES)))
    out = np.empty((B, NQ, T, DH), np.float32)
    for core in range(NCORES):
        b, kv = divmod(core, NKV)
        oc = res.results[core]["out"]
        for g in range(NREP):
            out[b, g * NKV + kv] = oc[g]
    return out


if __name__ == "__main__":
    _get_program()
    print("program built + compiled OK")
